# revision 1
# baseline (speedup 1.0000x reference)
"""HGT layer on 8 Trainium2 NeuronCores (Bass/Tile).

Strategy:
- dst-node-contiguous sharding: core c owns dst rows [2500c, 2500(c+1)) of both
  node types -> every edge lives on exactly one core, no cross-core softmax
  reduction needed.
- Host folds rel_att (and 1/sqrt(hd)) into Wk and rel_msg into Wv, so the
  per-edge relation transforms become plain per-node linear maps:
      ktmt = x_src @ [Wk_eff | Wv_eff]  (per relation, [N, 512])
- Each core computes its 1/8 chunk of ktmt, one AllGather builds the full
  table, then edges (host-sorted by dst, padded per 128-dst tile) are
  processed: indirect-DMA gather of ktmt rows, one-hot selection matrix S
  built on-device from dst offsets, q expanded per-edge via S^T @ q_window
  matmul, att = rowsum(kt*q) per head, w = exp(att + pri), and a single
  accumulated matmul S^T... lhsT=S aggregates both messages and softmax
  denominators into PSUM.
- amax trick: softmax is shift-invariant; att ~= 1 +- small here so exp() is
  computed without the per-segment max (matches reference to ~1e-6; the
  reference's max(.,0)/1e-8 clamps only matter for empty segments, handled by
  the same max(denom,1e-8)).
- Node FFN/LN is plain data-parallel dense compute on the owned 2500 rows.
"""
import os
import sys
import math
import numpy as np

sys.path.insert(0, "/opt/trn_rl_repo")

from contextlib import ExitStack

import concourse.bass as bass
import concourse.bacc as bacc
import concourse.tile as tile
import concourse.mybir as mybir
from concourse.bass_utils import run_bass_kernel_spmd
from concourse.masks import make_identity

H = 256
HEADS = 8
HD = 32
N = 20000
E = 320000
FF = 4 * H
CORES = 8
ND = N // CORES          # 2500 dst rows per core per type
NDT = (ND + 127) // 128  # 20 dst tiles per core (last tile 68 rows)
P = 128

f32 = mybir.dt.float32
bf16 = mybir.dt.bfloat16
i32 = mybir.dt.int32
OP = mybir.AluOpType
ACT = mybir.ActivationFunctionType
AX = mybir.AxisListType

_CACHE = {}


def _ln_tile(nc, pool, x_ap, g_sb, b_sb, out_tile):
    """LayerNorm over free dim (256) for a [128, 256] sbuf tile -> out_tile."""
    s1 = pool.tile([P, 1], f32, tag="ln_s1", name="ln_s1")
    nc.vector.reduce_sum(out=s1[:], in_=x_ap, axis=AX.X)
    mean = pool.tile([P, 1], f32, tag="ln_mean", name="ln_mean")
    nc.scalar.mul(mean[:], s1[:], 1.0 / H)
    xc = pool.tile([P, H], f32, tag="ln_xc", name="ln_xc")
    nc.vector.tensor_scalar(out=xc[:], in0=x_ap, scalar1=mean[:], scalar2=None,
                            op0=OP.subtract)
    sq = pool.tile([P, H], f32, tag="ln_sq", name="ln_sq")
    vs = pool.tile([P, 1], f32, tag="ln_vs", name="ln_vs")
    nc.scalar.activation(out=sq[:], in_=xc[:], func=ACT.Square, accum_out=vs[:])
    std = pool.tile([P, 1], f32, tag="ln_std", name="ln_std")
    nc.scalar.activation(out=std[:], in_=vs[:], func=ACT.Sqrt,
                         bias=_ln_tile.eps[:], scale=1.0 / H)
    rstd = pool.tile([P, 1], f32, tag="ln_rstd", name="ln_rstd")
    nc.vector.reciprocal(rstd[:], std[:])
    yn = pool.tile([P, H], f32, tag="ln_yn", name="ln_yn")
    nc.vector.tensor_scalar(out=yn[:], in0=xc[:], scalar1=rstd[:], scalar2=None,
                            op0=OP.mult)
    nc.vector.tensor_tensor(out=out_tile[:], in0=yn[:], in1=g_sb[:], op=OP.mult)
    nc.vector.tensor_tensor(out=out_tile[:], in0=out_tile[:], in1=b_sb[:],
                            op=OP.add)


def _transpose_pair(nc, pool, ppool, src_ap, ncols, tagbase):
    """Transpose [128, ncols*128] sbuf -> list of ncols [128,128] sbuf tiles."""
    outs = []
    for k in range(ncols):
        pt = ppool.tile([P, P], f32, tag="tp_pt", name="tp_pt", bufs=2)
        nc.tensor.transpose(pt[:], src_ap[:, k * P:(k + 1) * P],
                            _transpose_pair.ident[:])
        st = pool.tile([P, P], f32, tag=f"{tagbase}_{k}", name=f"{tagbase}_{k}")
        nc.vector.tensor_copy(out=st[:], in_=pt[:])
        outs.append(st)
    return outs


def build_program(tpd):
    """Build the SPMD Bass program. tpd = edge tiles (of 128) per dst tile."""
    T = NDT * tpd  # edge tiles per relation per core
    nc = bacc.Bacc("TRN2", target_bir_lowering=False, debug=False,
                   num_devices=CORES)

    def inp(name, shape, dt=f32):
        return nc.dram_tensor(name, shape, dt, kind="ExternalInput").ap()

    xa = inp("xa", [ND, H])
    xb = inp("xb", [ND, H])
    wkm = {r: inp(f"wkm_{r}", [H, 2 * H]) for r in ("ab", "ba")}
    bkm = {r: inp(f"bkm_{r}", [1, 2 * H]) for r in ("ab", "ba")}
    wq = {r: inp(f"wq_{r}", [H, H]) for r in ("ab", "ba")}
    bq = {r: inp(f"bq_{r}", [1, H]) for r in ("ab", "ba")}
    pri = {r: inp(f"pri_{r}", [P, HEADS]) for r in ("ab", "ba")}
    sidx = {r: inp(f"sidx_{r}", [P, T], i32) for r in ("ab", "ba")}
    dloc = {r: inp(f"dloc_{r}", [P, T]) for r in ("ab", "ba")}
    iota = inp("iota", [P, P])
    wo = {t: inp(f"wo_{t}", [H, H]) for t in "ab"}
    bo = {t: inp(f"bo_{t}", [1, H]) for t in "ab"}
    w1 = {t: inp(f"w1_{t}", [H, FF]) for t in "ab"}
    b1 = {t: inp(f"b1_{t}", [1, FF]) for t in "ab"}
    w2 = {t: inp(f"w2_{t}", [FF, H]) for t in "ab"}
    b2 = {t: inp(f"b2_{t}", [1, H]) for t in "ab"}
    ln = {}
    for t in "ab":
        for nm in ("ln1g", "ln1b", "ln2g", "ln2b"):
            ln[(t, nm)] = inp(f"{nm}_{t}", [P, H])
    out_d = {t: nc.dram_tensor(f"out_{t}", [ND, H], f32,
                               kind="ExternalOutput").ap() for t in "ab"}
    x_d = {"a": xa, "b": xb}

    with tile.TileContext(nc) as tc, ExitStack() as ctx:
        per = ctx.enter_context(tc.tile_pool(name="per", bufs=1))
        dram = ctx.enter_context(tc.tile_pool(name="dram", bufs=1, space="DRAM"))

        ident = per.tile([P, P], f32, tag="ident", name="ident")
        make_identity(nc, ident[:])
        _transpose_pair.ident = ident
        ones1 = per.tile([1, P], f32, tag="ones1", name="ones1")
        nc.vector.memset(ones1[:], 1.0)
        eps_sb = per.tile([P, 1], f32, tag="eps", name="eps")
        nc.vector.memset(eps_sb[:], 1e-5)
        _ln_tile.eps = eps_sb
        iota_sb = per.tile([P, P], f32, tag="iota", name="iota")
        nc.sync.dma_start(out=iota_sb[:], in_=iota)
        pri_sb, sidx_sb, dloc_sb = {}, {}, {}
        for r in ("ab", "ba"):
            pri_sb[r] = per.tile([P, HEADS], f32, tag=f"pri_{r}", name=f"pri_{r}")
            nc.sync.dma_start(out=pri_sb[r][:], in_=pri[r])
            sidx_sb[r] = per.tile([P, T], i32, tag=f"sidx_{r}", name=f"sidx_{r}")
            nc.sync.dma_start(out=sidx_sb[r][:], in_=sidx[r])
            dloc_sb[r] = per.tile([P, T], f32, tag=f"dloc_{r}", name=f"dloc_{r}")
            nc.sync.dma_start(out=dloc_sb[r][:], in_=dloc[r])
        qwin = {r: [per.tile([P, H], f32, tag=f"qw_{r}_{i}", name=f"qw_{r}_{i}") for i in range(NDT)]
                for r in ("ab", "ba")}
        mwin = {t: [per.tile([P, H], f32, tag=f"mw_{t}_{i}", name=f"mw_{t}_{i}") for i in range(NDT)]
                for t in "ab"}

        bounce = dram.tile([2 * ND, 2 * H], bf16, tag="bounce", name="bounce")
        table = dram.tile([2 * ND * CORES, 2 * H], bf16, tag="table", name="table")

        # ---- Phase A: per-core ktmt chunk + local q windows ----
        with tc.tile_pool(name="pa", bufs=3) as pa, \
             tc.tile_pool(name="pap", bufs=2, space="PSUM") as pap, \
             tc.tile_pool(name="paw", bufs=1) as paw:
            wkm_sb, bkm_sb, wq_sb, bq_sb = {}, {}, {}, {}
            for r in ("ab", "ba"):
                wkm_sb[r] = []
                wq_sb[r] = []
                for k in range(2):
                    wt = paw.tile([P, 2 * H], f32, tag=f"wkm_{r}{k}", name=f"wkm_{r}{k}")
                    nc.sync.dma_start(out=wt[:], in_=wkm[r][k * P:(k + 1) * P, :])
                    wkm_sb[r].append(wt)
                    qt = paw.tile([P, H], f32, tag=f"wq_{r}{k}", name=f"wq_{r}{k}")
                    nc.sync.dma_start(out=qt[:], in_=wq[r][k * P:(k + 1) * P, :])
                    wq_sb[r].append(qt)
                bkm_sb[r] = paw.tile([1, 2 * H], f32, tag=f"bkm_{r}", name=f"bkm_{r}")
                nc.sync.dma_start(out=bkm_sb[r][:], in_=bkm[r])
                bq_sb[r] = paw.tile([1, H], f32, tag=f"bq_{r}", name=f"bq_{r}")
                nc.sync.dma_start(out=bq_sb[r][:], in_=bq[r])

            for ti, (x_dram, kmr, qr) in enumerate(
                    [(xa, "ab", "ba"), (xb, "ba", "ab")]):
                for dt in range(NDT):
                    rows = min(P, ND - dt * P)
                    x_sb = pa.tile([P, H], f32, tag="x", name="x")
                    nc.sync.dma_start(out=x_sb[:rows],
                                      in_=x_dram[dt * P: dt * P + rows, :])
                    xT = _transpose_pair(nc, pa, pap, x_sb[:], 2, "xT")
                    pkm = pap.tile([P, 2 * H], f32, tag="pkm", name="pkm")
                    for k in range(2):
                        nc.tensor.matmul(pkm[:], lhsT=xT[k][:], rhs=wkm_sb[kmr][k][:],
                                         start=(k == 0), stop=False)
                    nc.tensor.matmul(pkm[:], lhsT=ones1[:], rhs=bkm_sb[kmr][:],
                                     start=False, stop=True)
                    km_sb = pa.tile([P, 2 * H], bf16, tag="km", name="km")
                    nc.vector.tensor_copy(out=km_sb[:], in_=pkm[:])
                    nc.sync.dma_start(
                        out=bounce[ti * ND + dt * P: ti * ND + dt * P + rows, :],
                        in_=km_sb[:rows])
                    pq = pap.tile([P, H], f32, tag="pq", name="pq")
                    for k in range(2):
                        nc.tensor.matmul(pq[:], lhsT=xT[k][:], rhs=wq_sb[qr][k][:],
                                         start=(k == 0), stop=False)
                    nc.tensor.matmul(pq[:], lhsT=ones1[:], rhs=bq_sb[qr][:],
                                     start=False, stop=True)
                    nc.vector.tensor_copy(out=qwin[qr][dt][:], in_=pq[:])

        # ---- AllGather the ktmt table ----
        nc.gpsimd.collective_compute(
            "AllGather", OP.bypass,
            ins=[bounce.opt()],
            outs=[table.opt()],
            replica_groups=[list(range(CORES))],
        )

        # ---- Phase B: edge processing ----
        _phases = os.environ.get("KPHASES", "ABC")
        for r, twin in ((("ab", "b"), ("ba", "a")) if "B" in _phases else ()):
            with tc.tile_pool(name=f"pb_{r}", bufs=4) as pb, \
                 tc.tile_pool(name=f"pbp_{r}", bufs=2, space="PSUM") as pbp:
                for dt in range(NDT):
                    pmsg = pbp.tile([P, H + HEADS], f32, tag="pmsg", name="pmsg")
                    for j in range(tpd):
                        t = dt * tpd + j
                        kg = pb.tile([P, 2 * H], bf16, tag="kg", name="kg", bufs=6)
                        nc.gpsimd.indirect_dma_start(
                            out=kg[:], out_offset=None,
                            in_=table[:, :],
                            in_offset=bass.IndirectOffsetOnAxis(
                                ap=sidx_sb[r][:, t:t + 1], axis=0),
                        )
                        S = pb.tile([P, P], f32, tag="S", name="S")
                        nc.vector.tensor_tensor(
                            out=S[:],
                            in0=dloc_sb[r][:, t:t + 1].to_broadcast([P, P]),
                            in1=iota_sb[:], op=OP.is_equal)
                        pst = pbp.tile([P, P], f32, tag="pst", name="pst")
                        nc.tensor.transpose(pst[:], S[:], ident[:])
                        St = pb.tile([P, P], f32, tag="St", name="St")
                        nc.vector.tensor_copy(out=St[:], in_=pst[:])
                        pqg = pbp.tile([P, H], f32, tag="pqg", name="pqg")
                        nc.tensor.matmul(pqg[:], lhsT=St[:], rhs=qwin[r][dt][:],
                                         start=True, stop=True)
                        prod = pb.tile([P, H], f32, tag="prod", name="prod")
                        nc.vector.tensor_tensor(out=prod[:], in0=kg[:, 0:H],
                                                in1=pqg[:], op=OP.mult)
                        att = pb.tile([P, HEADS], f32, tag="att", name="att")
                        nc.vector.reduce_sum(
                            out=att[:],
                            in_=prod[:].rearrange("p (h w) -> p h w", w=HD),
                            axis=AX.X)
                        att2 = pb.tile([P, HEADS], f32, tag="att2", name="att2")
                        nc.vector.tensor_tensor(out=att2[:], in0=att[:],
                                                in1=pri_sb[r][:], op=OP.add)
                        wmsg = pb.tile([P, H + HEADS], f32, tag="wmsg", name="wmsg")
                        nc.scalar.activation(out=wmsg[:, H:H + HEADS],
                                             in_=att2[:], func=ACT.Exp)
                        nc.vector.tensor_tensor(
                            out=wmsg[:, 0:H].rearrange("p (h w) -> p h w", w=HD),
                            in0=kg[:, H:2 * H].rearrange("p (h w) -> p h w", w=HD),
                            in1=wmsg[:, H:H + HEADS].to_broadcast([P, HEADS, HD]),
                            op=OP.mult)
                        nc.tensor.matmul(pmsg[:], lhsT=S[:], rhs=wmsg[:],
                                         start=(j == 0), stop=(j == tpd - 1))
                    den = pb.tile([P, HEADS], f32, tag="den", name="den")
                    nc.vector.tensor_scalar_max(den[:], pmsg[:, H:H + HEADS], 1e-8)
                    rec = pb.tile([P, HEADS], f32, tag="rec", name="rec")
                    nc.vector.reciprocal(rec[:], den[:])
                    nc.vector.tensor_tensor(
                        out=mwin[twin][dt][:].rearrange("p (h w) -> p h w", w=HD),
                        in0=pmsg[:, 0:H].rearrange("p (h w) -> p h w", w=HD),
                        in1=rec[:].to_broadcast([P, HEADS, HD]),
                        op=OP.mult)

        # ---- Phase C: node update (Wo, LN1, FFN, LN2) ----
        for t in ("ab" if "C" in _phases else ""):
            with tc.tile_pool(name=f"pc_{t}", bufs=3) as pc, \
                 tc.tile_pool(name=f"pcp_{t}", bufs=2, space="PSUM") as pcp, \
                 tc.tile_pool(name=f"pcw_{t}", bufs=1) as pcw:
                wo_sb = []
                w1_sb = []
                w2_sb = []
                for k in range(2):
                    wt = pcw.tile([P, H], f32, tag=f"wo{k}", name=f"wo{k}")
                    nc.sync.dma_start(out=wt[:], in_=wo[t][k * P:(k + 1) * P, :])
                    wo_sb.append(wt)
                    w1t = pcw.tile([P, FF], f32, tag=f"w1{k}", name=f"w1{k}")
                    nc.sync.dma_start(out=w1t[:], in_=w1[t][k * P:(k + 1) * P, :])
                    w1_sb.append(w1t)
                for k in range(8):
                    w2t = pcw.tile([P, H], f32, tag=f"w2{k}", name=f"w2{k}")
                    nc.sync.dma_start(out=w2t[:], in_=w2[t][k * P:(k + 1) * P, :])
                    w2_sb.append(w2t)
                bo_sb = pcw.tile([1, H], f32, tag="bo", name="bo")
                nc.sync.dma_start(out=bo_sb[:], in_=bo[t])
                b1_sb = pcw.tile([1, FF], f32, tag="b1", name="b1")
                nc.sync.dma_start(out=b1_sb[:], in_=b1[t])
                b2_sb = pcw.tile([1, H], f32, tag="b2", name="b2")
                nc.sync.dma_start(out=b2_sb[:], in_=b2[t])
                ln_sb = {}
                for nm in ("ln1g", "ln1b", "ln2g", "ln2b"):
                    lt = pcw.tile([P, H], f32, tag=nm)
                    nc.sync.dma_start(out=lt[:], in_=ln[(t, nm)])
                    ln_sb[nm] = lt

                for dt in range(NDT):
                    rows = min(P, ND - dt * P)
                    m = mwin[t][dt]
                    mT = _transpose_pair(nc, pc, pcp, m[:], 2, "mT")
                    po = pcp.tile([P, H], f32, tag="po", name="po", bufs=1)
                    for k in range(2):
                        nc.tensor.matmul(po[:], lhsT=mT[k][:], rhs=wo_sb[k][:],
                                         start=(k == 0), stop=False)
                    nc.tensor.matmul(po[:], lhsT=ones1[:], rhs=bo_sb[:],
                                     start=False, stop=True)
                    x_sb = pc.tile([P, H], f32, tag="x", name="x")
                    nc.sync.dma_start(out=x_sb[:rows],
                                      in_=x_d[t][dt * P: dt * P + rows, :])
                    r1 = pc.tile([P, H], f32, tag="r1", name="r1")
                    nc.vector.tensor_tensor(out=r1[:], in0=x_sb[:], in1=po[:],
                                            op=OP.add)
                    y1 = pc.tile([P, H], f32, tag="y1", name="y1")
                    _ln_tile(nc, pc, r1[:], ln_sb["ln1g"], ln_sb["ln1b"], y1)
                    y1T = _transpose_pair(nc, pc, pcp, y1[:], 2, "y1T")
                    ph = pcp.tile([P, FF], f32, tag="ph", name="ph", bufs=1)
                    for nn in range(2):
                        sl = slice(nn * 512, (nn + 1) * 512)
                        for k in range(2):
                            nc.tensor.matmul(ph[:, sl], lhsT=y1T[k][:],
                                             rhs=w1_sb[k][:, sl],
                                             start=(k == 0), stop=False)
                        nc.tensor.matmul(ph[:, sl], lhsT=ones1[:],
                                         rhs=b1_sb[:, sl], start=False, stop=True)
                    gh = pc.tile([P, FF], f32, tag="gh", name="gh")
                    nc.scalar.activation(out=gh[:], in_=ph[:], func=ACT.Gelu)
                    ghT = _transpose_pair(nc, pc, pcp, gh[:], 8, "ghT")
                    pz = pcp.tile([P, H], f32, tag="pz", name="pz", bufs=1)
                    for k in range(8):
                        nc.tensor.matmul(pz[:], lhsT=ghT[k][:], rhs=w2_sb[k][:],
                                         start=(k == 0), stop=False)
                    nc.tensor.matmul(pz[:], lhsT=ones1[:], rhs=b2_sb[:],
                                     start=False, stop=True)
                    r2 = pc.tile([P, H], f32, tag="r2", name="r2")
                    nc.vector.tensor_tensor(out=r2[:], in0=y1[:], in1=pz[:],
                                            op=OP.add)
                    y2 = pc.tile([P, H], f32, tag="y2", name="y2")
                    _ln_tile(nc, pc, r2[:], ln_sb["ln2g"], ln_sb["ln2b"], y2)
                    nc.sync.dma_start(out=out_d[t][dt * P: dt * P + rows, :],
                                      in_=y2[:rows])

    nc.compile()
    return nc


def _block_diag(rel):  # rel [HEADS, HD, HD] -> [H, H]
    out = np.zeros((H, H), np.float32)
    for h in range(HEADS):
        out[h * HD:(h + 1) * HD, h * HD:(h + 1) * HD] = rel[h]
    return out


def _prep_edges(ei, src_half, tpd):
    """Per-core edge tiles. Returns (sidx_cols[8], dloc_cols[8]) each [128, NDT*tpd]."""
    s = np.asarray(ei[0], np.int64)
    d = np.asarray(ei[1], np.int64)
    core = d // ND
    d_local = d - core * ND
    dt = d_local // P
    key = core * NDT + dt
    order = np.argsort(key, kind="stable")
    s, d_local, dt, key, core = s[order], d_local[order], dt[order], key[order], core[order]
    cnt = np.bincount(key, minlength=CORES * NDT)
    starts = np.concatenate([[0], np.cumsum(cnt)[:-1]])
    pos = np.arange(len(key)) - starts[key]
    srow = (2 * ND) * (s // ND) + (s % ND) + ND * src_half
    dval = (d_local - dt * P).astype(np.float32)
    ept = tpd * P
    sidx_arr = np.zeros((CORES, NDT, ept), np.int32)
    dloc_arr = np.full((CORES, NDT, ept), float(P), np.float32)
    sidx_arr[core, dt, pos] = srow
    dloc_arr[core, dt, pos] = dval
    sidx_cols = [np.ascontiguousarray(sidx_arr[c].reshape(NDT * tpd, P).T)
                 for c in range(CORES)]
    dloc_cols = [np.ascontiguousarray(dloc_arr[c].reshape(NDT * tpd, P).T)
                 for c in range(CORES)]
    return sidx_cols, dloc_cols


def _edge_tpd(ei_ab, ei_ba):
    mx = 0
    for ei in (ei_ab, ei_ba):
        d = np.asarray(ei[1], np.int64)
        core = d // ND
        dt = (d - core * ND) // P
        cnt = np.bincount(core * NDT + dt, minlength=CORES * NDT)
        mx = max(mx, int(cnt.max()))
    return (mx + P - 1) // P


LAST_RESULTS = None


def _prepare(inputs):
    inp = {k: np.asarray(v) for k, v in inputs.items()}
    x_a = inp["x_a"].astype(np.float32)
    x_b = inp["x_b"].astype(np.float32)
    scale = 1.0 / math.sqrt(HD)

    cfg = {}
    # relation ab: src a (st=0), et=0, dst b (dt=1); relation ba: mirrored
    for r, st, et, dtp in (("ab", 0, 0, 1), ("ba", 1, 1, 0)):
        bd_att = _block_diag(inp["rel_att"][et])
        bd_msg = _block_diag(inp["rel_msg"][et])
        wk_eff = (inp["Wk"][st] @ bd_att) * scale
        bk_eff = (inp["bk"][st] @ bd_att) * scale
        wv_eff = inp["Wv"][st] @ bd_msg
        bv_eff = inp["bv"][st] @ bd_msg
        cfg[f"wkm_{r}"] = np.ascontiguousarray(
            np.concatenate([wk_eff, wv_eff], 1).astype(np.float32))
        cfg[f"bkm_{r}"] = np.concatenate([bk_eff, bv_eff])[None, :].astype(np.float32)
        cfg[f"wq_{r}"] = np.ascontiguousarray(inp["Wq"][dtp].astype(np.float32))
        cfg[f"bq_{r}"] = inp["bq"][dtp][None, :].astype(np.float32)
        cfg[f"pri_{r}"] = np.tile(inp["rel_pri"][et][None, :], (P, 1)).astype(np.float32)
    for t, ti in (("a", 0), ("b", 1)):
        cfg[f"wo_{t}"] = np.ascontiguousarray(inp["Wo"][ti].astype(np.float32))
        cfg[f"bo_{t}"] = inp["bo"][ti][None, :].astype(np.float32)
        cfg[f"w1_{t}"] = np.ascontiguousarray(inp["W1"][ti].astype(np.float32))
        cfg[f"b1_{t}"] = inp["b1"][ti][None, :].astype(np.float32)
        cfg[f"w2_{t}"] = np.ascontiguousarray(inp["W2"][ti].astype(np.float32))
        cfg[f"b2_{t}"] = inp["b2"][ti][None, :].astype(np.float32)
        for nm, key in (("ln1g", "ln1_g"), ("ln1b", "ln1_b"),
                        ("ln2g", "ln2_g"), ("ln2b", "ln2_b")):
            cfg[f"{nm}_{t}"] = np.tile(inp[key][ti][None, :], (P, 1)).astype(np.float32)
    cfg["iota"] = np.tile(np.arange(P, dtype=np.float32)[None, :], (P, 1))

    tpd = _edge_tpd(inp["ei_ab"], inp["ei_ba"])
    sidx_ab, dloc_ab = _prep_edges(inp["ei_ab"], 0, tpd)
    sidx_ba, dloc_ba = _prep_edges(inp["ei_ba"], 1, tpd)

    if tpd not in _CACHE:
        _CACHE[tpd] = build_program(tpd)
    nc = _CACHE[tpd]

    in_maps = []
    for c in range(CORES):
        m = dict(cfg)
        m["xa"] = np.ascontiguousarray(x_a[c * ND:(c + 1) * ND])
        m["xb"] = np.ascontiguousarray(x_b[c * ND:(c + 1) * ND])
        m["sidx_ab"] = sidx_ab[c]
        m["dloc_ab"] = dloc_ab[c]
        m["sidx_ba"] = sidx_ba[c]
        m["dloc_ba"] = dloc_ba[c]
        in_maps.append(m)

    return nc, in_maps


def kernel(**inputs):
    global LAST_RESULTS
    nc, in_maps = _prepare(inputs)
    res = run_bass_kernel_spmd(nc, in_maps, core_ids=list(range(CORES)))
    LAST_RESULTS = res
    out_a = np.concatenate([res.results[c]["out_a"] for c in range(CORES)], 0)
    out_b = np.concatenate([res.results[c]["out_b"] for c in range(CORES)], 0)
    return out_a, out_b


def bench(inputs, iters=6):
    """Returns ((out_a, out_b), min_exec_seconds) timing only device execution."""
    import time
    import jax
    from jax.sharding import Mesh, PartitionSpec
    from jax.experimental.shard_map import shard_map
    from concourse import bass2jax, mybir as _mb

    nc, in_maps = _prepare(inputs)
    bass2jax.install_neuronx_cc_hook()
    in_names, out_names, out_avals, zero_outs = [], [], [], []
    for alloc in nc.m.functions[0].allocations:
        if not isinstance(alloc, _mb.MemoryLocationSet):
            continue
        nm = alloc.memorylocations[0].name
        pname = nc.partition_id_tensor.name if nc.partition_id_tensor else None
        if alloc.kind == "ExternalInput":
            if nm != pname:
                in_names.append(nm)
        elif alloc.kind == "ExternalOutput":
            out_names.append(nm)
            shape = tuple(alloc.tensor_shape)
            dtype = _mb.dt.np(alloc.dtype)
            out_avals.append(jax.core.ShapedArray(shape, dtype))
            zero_outs.append(np.zeros(shape, dtype))
    n_params = len(in_names)
    all_names = in_names + out_names
    pname = nc.partition_id_tensor.name if nc.partition_id_tensor else None
    if pname is not None:
        all_names = all_names + [pname]

    def _body(*args):
        operands = list(args)
        if pname is not None:
            operands.append(bass2jax.partition_id_tensor())
        return tuple(bass2jax._bass_exec_p.bind(
            *operands, out_avals=tuple(out_avals), in_names=tuple(all_names),
            out_names=tuple(out_names), lowering_input_output_aliases=(),
            sim_require_finite=True, sim_require_nnan=True, nc=nc))

    devices = jax.devices()[:CORES]
    mesh = Mesh(np.asarray(devices), ("core",))
    donate = tuple(range(n_params, n_params + len(out_names)))
    sharded = jax.jit(
        shard_map(_body, mesh=mesh,
                  in_specs=(PartitionSpec("core"),) * (n_params + len(out_names)),
                  out_specs=(PartitionSpec("core"),) * len(out_names),
                  check_rep=False),
        donate_argnums=donate, keep_unused=True)
    concat_in = [np.concatenate([np.asarray(in_maps[c][nm]) for c in range(CORES)], 0)
                 for nm in in_names]
    concat_in = [jax.device_put(x) for x in concat_in]
    best = None
    out_arrs = None
    for _ in range(iters):
        zeros = [jax.device_put(np.zeros((CORES * z.shape[0], *z.shape[1:]), z.dtype))
                 for z in zero_outs]
        jax.block_until_ready(zeros)
        t0 = time.perf_counter()
        out_arrs = sharded(*concat_in, *zeros)
        jax.block_until_ready(out_arrs)
        dt = time.perf_counter() - t0
        best = dt if best is None else min(best, dt)
    outs = {nm: np.asarray(out_arrs[i]).reshape(CORES, *out_avals[i].shape)
            for i, nm in enumerate(out_names)}
    out_a = outs["out_a"].reshape(N, H)
    out_b = outs["out_b"].reshape(N, H)
    return (out_a, out_b), best

